# revision 40
# baseline (speedup 1.0000x reference)
"""Dual-stream BERT self-attention (B=4, S=1024, H=12, DH=64) on 8 Trainium2
NeuronCores.

Sharding: core c handles batch b = c // 2 and a block of 6 heads
(hh = c % 2).  Each core computes its six QKV-style projections, the two
score blocks, a single softmax over the 2048 concatenated keys, and the
probability-weighted value sum for its (batch, head-block) slice.  No
cross-core communication; the host reassembles the [4, 1024, 768] output
from the per-core [1024, 384] slices.

All matmul data is fp16 (same 1 PE-cycle/row as fp32r, half the DMA and
SBUF traffic; fp8 DoubleRow would halve PE time but its ~4-7% quantization
noise on q/k/probabilities/values exceeds the 2e-2 tolerance by an order
of magnitude on concentrated softmax queries).  The contraction over
D=768 runs in six 128-deep chunks; linear biases are applied during the
PSUM->SBUF copy (per-partition bias for the transposed q/k layouts, a
broadcast row for v) instead of burning a seventh contraction chunk.

Engine plan: the Activation engine does nothing but the 96 exp
instructions ([128 keys, 1024 queries] each, mask riding in the
per-partition bias, 1/sqrt(64) in the scale, fp16 output) — they total
~100 us, hidden under the PE's ~130 us of matmuls.  All PSUM->SBUF
copies, bias adds, reciprocals and normalization muls run on the Vector
engine (GpSimd cannot access PSUM on TRN2); input DMAs split across the
SP and ACT HWDGE rings, with the ACT-side triggers all issued during its
pre-softmax idle window.  Within each attention block, chunk c's score
matmuls are emitted before chunk c-1's PV matmuls so the PE rides one
chunk ahead of the exp stream, and the other-stream value projection,
the next pair's transposed projections, and the previous pair's
transpose/normalize tail are interleaved into the chunk loop as PE
filler work while the Activation engine drains its exp backlog.

Softmax needs no max pass: scores*scale are ~N(0,1.4), so exp() spans
~[e-9, e+9], comfortably inside fp16 range.  The softmax denominator
rides as a ones column in the value matrix, so one accumulated matmul
yields both context and normalizer; a PE transpose + per-partition
reciprocal-multiply normalizes into the output staging tiles.
"""

import numpy as np

import concourse.bass as bass
import concourse.tile as tile
import concourse.mybir as mybir
from concourse.bass_utils import run_bass_kernel_spmd

F32 = mybir.dt.float32
F16 = mybir.dt.float16
AF = mybir.ActivationFunctionType
ALU = mybir.AluOpType

B, S, D = 4, 1024, 768
H, DH = 12, 64
HPC = 6           # heads per core
MW = HPC * DH     # 384: per-core projection width
N_CORES = 8
KC = 6            # contraction chunks of 128 over D=768
SC = S // 128     # 8 s-chunks
NQ = 2            # 512-wide query halves
KCH = 2 * SC      # 16 key chunks (self ++ other)

_DMA_OPCODES = {"DMACopy", "DMATranspose", "Trigger"}


def _split_sync_commands(nc, max_waits=1, max_updates=1):
    """This container's walrus accepts at most one sync-wait and one
    sync-update per instruction; split extras onto same-engine nops."""
    n = [0]

    def mknop(engine, waits, updates):
        n[0] += 1
        return mybir.InstNoOp(
            name=f"syncsplit-{n[0]}",
            engine=engine,
            bass_nofuse=True,
            sync_info=mybir.SyncInfo(on_wait=waits, on_update=updates),
        )

    for f in nc.m.functions:
        for bb in f.blocks:
            out = []
            changed = False
            for inst in bb.instructions:
                si = getattr(inst, "sync_info", None)
                if si is None:
                    out.append(inst)
                    continue
                waits = list(si.on_wait or [])
                if len(waits) > max_waits:
                    changed = True
                    si.on_wait = waits[:max_waits]
                    for i in range(max_waits, len(waits), max_waits):
                        out.append(mknop(inst.engine, waits[i:i + max_waits], []))
                out.append(inst)
                ups = list(si.on_update or [])
                if len(ups) > max_updates:
                    assert inst.opcode not in _DMA_OPCODES, (
                        f"can't split updates on async op {inst.name}"
                    )
                    changed = True
                    si.on_update = ups[:max_updates]
                    for i in range(max_updates, len(ups), max_updates):
                        out.append(mknop(inst.engine, [], ups[i:i + max_updates]))
            if changed:
                bb.instructions[:] = out


class CompatTileContext(tile.TileContext):
    def __exit__(self, exc_type, exc_val, exc_tb):
        r = super().__exit__(exc_type, exc_val, exc_tb)
        if exc_type is None:
            _split_sync_commands(self.nc)
        return r


def _build(repeat=1):
    nc = bass.Bass("TRN2", target_bir_lowering=False, debug=False,
                   enable_asserts=True, num_devices=1)

    xt_d = nc.dram_tensor("xt", [128, KC * S], F16, kind="ExternalInput").ap()
    xot_d = nc.dram_tensor("xot", [128, KC * S], F16, kind="ExternalInput").ap()
    w_d = {
        ty: nc.dram_tensor(f"w{ty}", [128, KC * MW], F16, kind="ExternalInput").ap()
        for ty in ("q", "k", "qo", "ko", "v", "vo")
    }
    pbias_d = nc.dram_tensor("pbias", [128, 12], F32, kind="ExternalInput").ap()
    vbias_d = nc.dram_tensor("vbias", [128, 2 * MW], F32,
                             kind="ExternalInput").ap()
    mact_d = nc.dram_tensor("mact", [128, SC], F32, kind="ExternalInput").ap()
    mzero_d = nc.dram_tensor("mzero", [128, 1], F32, kind="ExternalInput").ap()
    eye_d = nc.dram_tensor("eye", [128, 128], F32, kind="ExternalInput").ap()
    out_d = nc.dram_tensor("out", [S, MW], F32, kind="ExternalOutput").ap()

    ty_idx = {"q": 0, "k": 1, "qo": 2, "ko": 3}

    with CompatTileContext(nc) as tc:
      for _rep in range(repeat):
        with (
            tc.tile_pool(name="io", bufs=1) as io,
            tc.tile_pool(name="proj", bufs=1) as proj,
            tc.tile_pool(name="outp", bufs=1) as outp,
            tc.tile_pool(name="sps", bufs=2, space="PSUM") as sps,
            tc.tile_pool(name="pvs", bufs=2, space="PSUM") as pvs,
            tc.tile_pool(name="expp", bufs=6) as expp,
            tc.tile_pool(name="ctxp", bufs=4) as ctxp,
            tc.tile_pool(name="smal", bufs=8) as smal,
        ):
            # Bulk inputs in first-use order, split over both HWDGE rings:
            # SP carries the self-stream (first compute phase), the ACT ring
            # carries the other-stream (ACT is idle until the first exp).
            xt = io.tile([128, KC, S], F16, tag="xt")
            xot = io.tile([128, KC, S], F16, tag="xot")
            w_t = {
                ty: io.tile([128, KC, MW], F16, tag=f"w{ty}", name=f"w_{ty}")
                for ty in ("q", "k", "qo", "ko", "v", "vo")
            }
            nc.sync.dma_start(
                w_t["q"][:].rearrange("p a b -> p (a b)"), w_d["q"][:])
            nc.scalar.dma_start(
                w_t["k"][:].rearrange("p a b -> p (a b)"), w_d["k"][:])
            for k in range(KC):
                nc.sync.dma_start(xt[:, k, :], xt_d[:, S * k: S * k + S])
            pbias_t = io.tile([128, 12], F32, tag="pbias")
            nc.scalar.dma_start(pbias_t[:], pbias_d[:])
            for ty in ("qo", "ko"):
                nc.scalar.dma_start(
                    w_t[ty][:].rearrange("p a b -> p (a b)"), w_d[ty][:])
            for k in range(KC):
                nc.scalar.dma_start(xot[:, k, :], xot_d[:, S * k: S * k + S])
            nc.sync.dma_start(
                w_t["v"][:].rearrange("p a b -> p (a b)"), w_d["v"][:])
            vbias_t = io.tile([128, 2, HPC, DH], F32, tag="vbias")
            nc.sync.dma_start(
                vbias_t[:].rearrange("p a h d -> p (a h d)"), vbias_d[:])
            nc.scalar.dma_start(
                w_t["vo"][:].rearrange("p a b -> p (a b)"), w_d["vo"][:])
            mact_t = io.tile([128, SC], F32, tag="mact")
            nc.sync.dma_start(mact_t[:], mact_d[:])
            mzero_t = io.tile([128, 1], F32, tag="mzero")
            nc.sync.dma_start(mzero_t[:], mzero_d[:])
            eye_t = io.tile([128, 128], F32, tag="eye")
            nc.sync.dma_start(eye_t[:], eye_d[:])

            # allv[p, head, chunk, dh|1]: value rows for the fused
            # context+denominator matmul (ones column last).
            allv = proj.tile([128, HPC, KCH, DH + 1], F16, tag="av")
            nc.vector.memset(allv[:, :, :, DH:DH + 1], 1.0)

            # qt/kt/qot/kot: transposed projections [dout, s], 2 heads/tile
            pt = {
                ty: [proj.tile([128, S], F16, tag=f"{ty}{p}", name=f"pt_{ty}{p}")
                     for p in range(3)]
                for ty in ("q", "k", "qo", "ko")
            }

            out_sb = [[outp.tile([128, 128], F32, tag=f"o{p}_{qc}",
                                 name=f"out_sb{p}_{qc}") for qc in range(SC)]
                      for p in range(3)]


            # ---- Values: natural layout [s, 6*dh], all heads ------------
            def projV_group(ti, sc):
                src = xt if ti == 0 else xot
                w = w_t["v" if ti == 0 else "vo"]
                ps = sps.tile([128, S], F32, tag="sc", name=f"vps_{ti}{sc}")
                for k in range(KC):
                    nc.tensor.matmul(
                        ps[:, 0:MW],
                        src[:, k, 128 * sc: 128 * sc + 128],
                        w[:, k, :],
                        start=(k == 0), stop=(k == KC - 1),
                    )
                nc.vector.scalar_tensor_tensor(
                    allv[:, :, SC * ti + sc, 0:DH],
                    ps[:, 0:MW].rearrange("p (h d) -> p h d", d=DH),
                    1.0,
                    vbias_t[:, ti, :, :],
                    ALU.mult, ALU.add,
                )

            def projV(ti):
                for sc in range(SC):
                    projV_group(ti, sc)

            # ---- Transposed projections for pair p of type ty -----------
            def projT_group(ty, p, nh):
                src = xot if ty == "ko" else xt
                w = w_t[ty]
                ps = sps.tile([128, S], F32, tag="sc", name=f"ps_{ty}{p}{nh}")
                for k in range(KC):
                    nc.tensor.matmul(
                        ps[:, 0:512],
                        w[:, k, 128 * p: 128 * p + 128],
                        src[:, k, 512 * nh: 512 * nh + 512],
                        start=(k == 0), stop=(k == KC - 1),
                    )
                bias_ap = pbias_t[:, 3 * ty_idx[ty] + p:
                                  3 * ty_idx[ty] + p + 1]
                nc.vector.tensor_scalar(
                    pt[ty][p][:, 512 * nh: 512 * nh + 512],
                    ps[:, 0:512], bias_ap, None, ALU.add)

            def projT(ty, p):
                for nh in range(NQ):
                    projT_group(ty, p, nh)

            # ---- Attention for the two heads of pair p ------------------
            # fillers: next pair's projection psum-groups, interleaved into
            # the chunk loop so the PE keeps working while the (slower)
            # Activation engine drains its exp backlog.
            def attention(p, fillers=()):
                fillers = list(fillers)
                h0, h1 = 2 * p, 2 * p + 1
                pv0 = pvs.tile([DH + 1, S], F32, tag="pv", name=f"pv{h0}")
                pv1 = pvs.tile([DH + 1, S], F32, tag="pv", name=f"pv{h1}")

                def emit_pv(c, et0, et1):
                    for pv, h, et in ((pv0, h0, et0), (pv1, h1, et1)):
                        for nh in range(NQ):
                            nc.tensor.matmul(
                                pv[:, 512 * nh: 512 * nh + 512],
                                allv[:, h, c, :],
                                et[:, 512 * nh: 512 * nh + 512],
                                start=(c == 0), stop=(c == KCH - 1),
                            )

                prev = None
                for c in range(KCH):
                    self_side = c < SC
                    kt_src = pt["k" if self_side else "ko"][p]
                    qt_src = pt["q" if self_side else "qo"][p]
                    col = 128 * (c % SC)
                    sc0 = sps.tile([128, S], F32, tag="sc", name=f"sc{h0}_{c}")
                    sc1 = sps.tile([128, S], F32, tag="sc", name=f"sc{h1}_{c}")
                    # Adjacent K=64 matmuls on row groups 0 / 64 overlap in
                    # the PE array.
                    for nh in range(NQ):
                        nc.tensor.matmul(
                            sc0[:, 512 * nh: 512 * nh + 512],
                            kt_src[0:64, col:col + 128],
                            qt_src[0:64, 512 * nh: 512 * nh + 512],
                            start=True, stop=True,
                        )
                        nc.tensor.matmul(
                            sc1[:, 512 * nh: 512 * nh + 512],
                            kt_src[64:128, col:col + 128],
                            qt_src[64:128, 512 * nh: 512 * nh + 512],
                            start=True, stop=True,
                        )
                    bias = (mact_t[:, (c % SC):(c % SC) + 1] if self_side
                            else mzero_t[:])
                    et0 = expp.tile([128, S], F16, tag="et", name=f"et{h0}_{c}")
                    nc.scalar.activation(et0[:], sc0[:], AF.Exp, bias=bias,
                                         scale=0.125)
                    et1 = expp.tile([128, S], F16, tag="et", name=f"et{h1}_{c}")
                    nc.scalar.activation(et1[:], sc1[:], AF.Exp, bias=bias,
                                         scale=0.125)
                    if fillers and c >= 1:
                        fillers.pop(0)()
                    # PV of the previous chunk: the PE waits on exp(c-1) only
                    # after issuing chunk c's scores, absorbing exp latency.
                    if prev is not None:
                        emit_pv(*prev)
                    prev = (c, et0, et1)
                emit_pv(*prev)
                for f in fillers:
                    f()
                cts = []
                for h, pv in ((h0, pv0), (h1, pv1)):
                    ct = ctxp.tile([DH + 1, S], F32, tag="ct", name=f"ct{h}")
                    nc.vector.tensor_copy(ct[:], pv[:])
                    cts.append((h, ct))

                def tail():
                    for h, ct in cts:
                        for qc in range(SC):
                            tp = sps.tile([128, DH + 1], F32, tag="sc",
                                          name=f"tp{h}_{qc}")
                            nc.tensor.transpose(
                                tp[:], ct[:, 128 * qc: 128 * qc + 128],
                                eye_t[0:DH + 1, 0:DH + 1],
                            )
                            rec = smal.tile([128, 1], F32, tag="rec",
                                            name=f"rec{h}_{qc}")
                            nc.vector.reciprocal(rec[:], tp[:, DH:DH + 1])
                            nc.vector.tensor_scalar_mul(
                                out_sb[p][qc][:, DH * (h % 2):
                                              DH * (h % 2) + DH],
                                tp[:, 0:DH], rec[:])
                    for qc in range(SC):
                        nc.sync.dma_start(
                            out_d[128 * qc: 128 * qc + 128,
                                  128 * p: 128 * p + 128],
                            out_sb[p][qc][:])
                return tail

            def pair_fillers(p):
                return [
                    (lambda ty=ty, nh=nh: projT_group(ty, p, nh))
                    for ty in ("q", "k", "qo", "ko") for nh in range(NQ)
                ]

            projT("q", 0)
            projT("k", 0)
            projV(0)
            projT("qo", 0)
            projT("ko", 0)
            # The other-stream value projection and the later pairs'
            # transposed projections ride as attention fillers: vo chunk j
            # lands at filler slot j+1, well before PV chunk 8+j reads it.
            vo_fillers = [(lambda sc=sc: projV_group(1, sc))
                          for sc in range(SC)]
            t0 = attention(0, vo_fillers + pair_fillers(1))
            t1 = attention(1, [t0] + pair_fillers(2))
            t2 = attention(2, [t1])
            t2()

    return nc


def _to_chunked(a, ncols):
    """[KC*128, ncols] -> [128, KC*ncols] with chunk c at cols [c*ncols, ...)."""
    return np.ascontiguousarray(
        a.reshape(KC, 128, ncols).transpose(1, 0, 2).reshape(128, KC * ncols)
    )


def _shard_inputs(hidden_states, hidden_states_other, attention_mask,
                  Wq, bq, Wk, bk, Wv, bv, Wqo, bqo, Wko, bko, Wvo, bvo):
    f32, f16 = np.float32, np.float16
    hs = np.asarray(hidden_states, f32)
    hso = np.asarray(hidden_states_other, f32)
    am = np.asarray(attention_mask, f32)
    ws = {"q": (Wq, bq), "k": (Wk, bk), "qo": (Wqo, bqo), "ko": (Wko, bko),
          "v": (Wv, bv), "vo": (Wvo, bvo)}
    ws = {ty: (np.asarray(w, f32), np.asarray(b, f32))
          for ty, (w, b) in ws.items()}

    eye = np.eye(128, dtype=f32)
    mzero = np.zeros((128, 1), f32)

    in_maps = []
    for core in range(N_CORES):
        b, hh = core // 2, core % 2
        m = {
            "xt": _to_chunked(hs[b].T, S).astype(f16),
            "xot": _to_chunked(hso[b].T, S).astype(f16),
        }
        sl = slice(MW * hh, MW * hh + MW)
        pbias = np.zeros((128, 12), f32)
        for ty, (W, bias) in ws.items():
            m[f"w{ty}"] = _to_chunked(W[sl].T, MW).astype(f16)
            if ty in ("q", "k", "qo", "ko"):
                col0 = 3 * {"q": 0, "k": 1, "qo": 2, "ko": 3}[ty]
                for p in range(3):
                    pbias[:, col0 + p] = bias[sl][128 * p: 128 * p + 128]
        m["pbias"] = pbias
        vb = np.concatenate([ws["v"][1][sl], ws["vo"][1][sl]])
        m["vbias"] = np.broadcast_to(vb, (128, 2 * MW)).copy()
        m["mact"] = np.ascontiguousarray(am[b, 0, 0].reshape(SC, 128).T)
        m["mzero"] = mzero
        m["eye"] = eye
        in_maps.append(m)
    return in_maps


_NC_CACHE = {}


def _get_nc(repeat=1):
    if repeat not in _NC_CACHE:
        _NC_CACHE[repeat] = _build(repeat)
    return _NC_CACHE[repeat]


def kernel(**inputs):
    in_maps = _shard_inputs(**inputs)
    nc = _get_nc()
    res = run_bass_kernel_spmd(nc, in_maps, core_ids=list(range(N_CORES)))
    out = np.empty((B, S, D), np.float32)
    for core in range(N_CORES):
        b, hh = core // 2, core % 2
        out[b, :, MW * hh:MW * hh + MW] = res.results[core]["out"]
    return out


# revision 42
# speedup vs baseline: 1.3132x; 1.3132x over previous
"""Dual-stream BERT self-attention (B=4, S=1024, H=12, DH=64) on 8 Trainium2
NeuronCores.

Sharding: core c handles batch b = c // 2 and a block of 6 heads
(hh = c % 2).  Each core computes its six QKV-style projections, the two
score blocks, a single softmax over the 2048 concatenated keys, and the
probability-weighted value sum for its (batch, head-block) slice.  No
cross-core communication; the host reassembles the [4, 1024, 768] output
from the per-core [1024, 384] slices.

All matmul data is fp16 (same 1 PE-cycle/row as fp32r, half the DMA and
SBUF traffic; fp8 DoubleRow would halve PE time but its ~4-7% quantization
noise on q/k/probabilities/values exceeds the 2e-2 tolerance by an order
of magnitude on concentrated softmax queries).  The contraction over
D=768 runs in six 128-deep chunks; linear biases are applied during the
PSUM->SBUF copy (per-partition bias for the transposed q/k layouts, a
broadcast row for v) instead of burning a seventh contraction chunk.

Engine plan: the Activation engine does nothing but the 96 exp
instructions ([128 keys, 1024 queries] each, mask riding in the
per-partition bias, 1/sqrt(64) in the scale, fp16 output) — they total
~100 us, hidden under the PE's ~130 us of matmuls.  All PSUM->SBUF
copies, bias adds, reciprocals and normalization muls run on the Vector
engine (GpSimd cannot access PSUM on TRN2); input DMAs split across the
SP and ACT HWDGE rings, with the ACT-side triggers all issued during its
pre-softmax idle window.  Within each attention block, chunk c's score
matmuls are emitted before chunk c-1's PV matmuls so the PE rides one
chunk ahead of the exp stream, and the other-stream value projection,
the next pair's transposed projections, and the previous pair's
transpose/normalize tail are interleaved into the chunk loop as PE
filler work while the Activation engine drains its exp backlog.

Softmax needs no max pass: scores*scale are ~N(0,1.4), so exp() spans
~[e-9, e+9], comfortably inside fp16 range.  The softmax denominator
rides as a ones column in the value matrix, so one accumulated matmul
yields both context and normalizer; a PE transpose + per-partition
reciprocal-multiply normalizes into the output staging tiles.
"""

import numpy as np

import concourse.bass as bass
import concourse.tile as tile
import concourse.mybir as mybir
from concourse.bass_utils import run_bass_kernel_spmd

F32 = mybir.dt.float32
F16 = mybir.dt.float16
AF = mybir.ActivationFunctionType
ALU = mybir.AluOpType

B, S, D = 4, 1024, 768
H, DH = 12, 64
HPC = 6           # heads per core
MW = HPC * DH     # 384: per-core projection width
N_CORES = 8
KC = 6            # contraction chunks of 128 over D=768
SC = S // 128     # 8 s-chunks
NQ = 2            # 512-wide query halves
KCH = 2 * SC      # 16 key chunks (self ++ other)

_DMA_OPCODES = {"DMACopy", "DMATranspose", "Trigger"}


def _split_sync_commands(nc, max_waits=1, max_updates=1):
    """This container's walrus accepts at most one sync-wait and one
    sync-update per instruction; split extras onto same-engine nops."""
    n = [0]

    def mknop(engine, waits, updates):
        n[0] += 1
        return mybir.InstNoOp(
            name=f"syncsplit-{n[0]}",
            engine=engine,
            bass_nofuse=True,
            sync_info=mybir.SyncInfo(on_wait=waits, on_update=updates),
        )

    for f in nc.m.functions:
        for bb in f.blocks:
            out = []
            changed = False
            for inst in bb.instructions:
                si = getattr(inst, "sync_info", None)
                if si is None:
                    out.append(inst)
                    continue
                waits = list(si.on_wait or [])
                if len(waits) > max_waits:
                    changed = True
                    si.on_wait = waits[:max_waits]
                    for i in range(max_waits, len(waits), max_waits):
                        out.append(mknop(inst.engine, waits[i:i + max_waits], []))
                out.append(inst)
                ups = list(si.on_update or [])
                if len(ups) > max_updates:
                    assert inst.opcode not in _DMA_OPCODES, (
                        f"can't split updates on async op {inst.name}"
                    )
                    changed = True
                    si.on_update = ups[:max_updates]
                    for i in range(max_updates, len(ups), max_updates):
                        out.append(mknop(inst.engine, [], ups[i:i + max_updates]))
            if changed:
                bb.instructions[:] = out


class CompatTileContext(tile.TileContext):
    def __exit__(self, exc_type, exc_val, exc_tb):
        r = super().__exit__(exc_type, exc_val, exc_tb)
        if exc_type is None:
            _split_sync_commands(self.nc)
        return r


def _build(repeat=1):
    nc = bass.Bass("TRN2", target_bir_lowering=False, debug=False,
                   enable_asserts=True, num_devices=1)

    xt_d = nc.dram_tensor("xt", [128, KC * S], F16, kind="ExternalInput").ap()
    xot_d = nc.dram_tensor("xot", [128, KC * S], F16, kind="ExternalInput").ap()
    w_d = {
        ty: nc.dram_tensor(f"w{ty}", [128, KC * MW], F16, kind="ExternalInput").ap()
        for ty in ("q", "k", "qo", "ko", "v", "vo")
    }
    pbias_d = nc.dram_tensor("pbias", [128, 12], F32, kind="ExternalInput").ap()
    vbias_d = nc.dram_tensor("vbias", [128, 2 * MW], F32,
                             kind="ExternalInput").ap()
    mact_d = nc.dram_tensor("mact", [128, SC], F32, kind="ExternalInput").ap()
    mzero_d = nc.dram_tensor("mzero", [128, 1], F32, kind="ExternalInput").ap()
    eye_d = nc.dram_tensor("eye", [128, 128], F16, kind="ExternalInput").ap()
    out_d = nc.dram_tensor("out", [S, MW], F32, kind="ExternalOutput").ap()

    ty_idx = {"q": 0, "k": 1, "qo": 2, "ko": 3}

    with CompatTileContext(nc) as tc:
      for _rep in range(repeat):
        with (
            tc.tile_pool(name="io", bufs=1) as io,
            tc.tile_pool(name="proj", bufs=1) as proj,
            tc.tile_pool(name="outp", bufs=1) as outp,
            tc.tile_pool(name="sps", bufs=2, space="PSUM") as sps,
            tc.tile_pool(name="pvs", bufs=2, space="PSUM") as pvs,
            tc.tile_pool(name="expp", bufs=6) as expp,
            tc.tile_pool(name="ctxp", bufs=4) as ctxp,
            tc.tile_pool(name="smal", bufs=8) as smal,
        ):
            # Bulk inputs in first-use order, split over both HWDGE rings:
            # SP carries the self-stream (first compute phase), the ACT ring
            # carries the other-stream (ACT is idle until the first exp).
            xt = io.tile([128, KC, S], F16, tag="xt")
            xot = io.tile([128, KC, S], F16, tag="xot")
            w_t = {
                ty: io.tile([128, KC, MW], F16, tag=f"w{ty}", name=f"w_{ty}")
                for ty in ("q", "k", "qo", "ko", "v", "vo")
            }
            nc.sync.dma_start(
                w_t["q"][:].rearrange("p a b -> p (a b)"), w_d["q"][:])
            nc.scalar.dma_start(
                w_t["k"][:].rearrange("p a b -> p (a b)"), w_d["k"][:])
            for k in range(KC):
                nc.sync.dma_start(xt[:, k, :], xt_d[:, S * k: S * k + S])
            pbias_t = io.tile([128, 12], F32, tag="pbias")
            nc.scalar.dma_start(pbias_t[:], pbias_d[:])
            for ty in ("qo", "ko"):
                nc.scalar.dma_start(
                    w_t[ty][:].rearrange("p a b -> p (a b)"), w_d[ty][:])
            for k in range(KC):
                nc.scalar.dma_start(xot[:, k, :], xot_d[:, S * k: S * k + S])
            nc.sync.dma_start(
                w_t["v"][:].rearrange("p a b -> p (a b)"), w_d["v"][:])
            vbias_t = io.tile([128, 2, HPC, DH], F32, tag="vbias")
            nc.sync.dma_start(
                vbias_t[:].rearrange("p a h d -> p (a h d)"), vbias_d[:])
            nc.scalar.dma_start(
                w_t["vo"][:].rearrange("p a b -> p (a b)"), w_d["vo"][:])
            mact_t = io.tile([128, SC], F32, tag="mact")
            nc.sync.dma_start(mact_t[:], mact_d[:])
            mzero_t = io.tile([128, 1], F32, tag="mzero")
            nc.sync.dma_start(mzero_t[:], mzero_d[:])
            eye_t = io.tile([128, 128], F16, tag="eye")
            nc.sync.dma_start(eye_t[:], eye_d[:])

            # allv[p, head, chunk, dh|1]: value rows for the fused
            # context+denominator matmul (ones column last).
            allv = proj.tile([128, HPC, KCH, DH + 1], F16, tag="av")
            nc.vector.memset(allv[:, :, :, DH:DH + 1], 1.0)

            # qt/kt/qot/kot: transposed projections [dout, s], 2 heads/tile
            pt = {
                ty: [proj.tile([128, S], F16, tag=f"{ty}{p}", name=f"pt_{ty}{p}")
                     for p in range(3)]
                for ty in ("q", "k", "qo", "ko")
            }

            out_sb = [[outp.tile([128, 128], F32, tag=f"o{p}_{qc}",
                                 name=f"out_sb{p}_{qc}") for qc in range(SC)]
                      for p in range(3)]


            # ---- Values: natural layout [s, 6*dh], all heads ------------
            def projV_group(ti, sc):
                src = xt if ti == 0 else xot
                w = w_t["v" if ti == 0 else "vo"]
                ps = sps.tile([128, S], F32, tag="sc", name=f"vps_{ti}{sc}")
                for k in range(KC):
                    nc.tensor.matmul(
                        ps[:, 0:MW],
                        src[:, k, 128 * sc: 128 * sc + 128],
                        w[:, k, :],
                        start=(k == 0), stop=(k == KC - 1),
                    )
                nc.vector.scalar_tensor_tensor(
                    allv[:, :, SC * ti + sc, 0:DH],
                    ps[:, 0:MW].rearrange("p (h d) -> p h d", d=DH),
                    1.0,
                    vbias_t[:, ti, :, :],
                    ALU.mult, ALU.add,
                )

            def projV(ti):
                for sc in range(SC):
                    projV_group(ti, sc)

            # ---- Transposed projections for pair p of type ty -----------
            def projT_group(ty, p, nh):
                src = xot if ty == "ko" else xt
                w = w_t[ty]
                ps = sps.tile([128, S], F32, tag="sc", name=f"ps_{ty}{p}{nh}")
                for k in range(KC):
                    nc.tensor.matmul(
                        ps[:, 0:512],
                        w[:, k, 128 * p: 128 * p + 128],
                        src[:, k, 512 * nh: 512 * nh + 512],
                        start=(k == 0), stop=(k == KC - 1),
                    )
                bias_ap = pbias_t[:, 3 * ty_idx[ty] + p:
                                  3 * ty_idx[ty] + p + 1]
                nc.vector.tensor_scalar(
                    pt[ty][p][:, 512 * nh: 512 * nh + 512],
                    ps[:, 0:512], bias_ap, None, ALU.add)

            def projT(ty, p):
                for nh in range(NQ):
                    projT_group(ty, p, nh)

            # ---- Attention for the two heads of pair p ------------------
            # fillers: next pair's projection psum-groups, interleaved into
            # the chunk loop so the PE keeps working while the (slower)
            # Activation engine drains its exp backlog.
            def attention(p, fillers=()):
                fillers = list(fillers)
                h0, h1 = 2 * p, 2 * p + 1
                pv0 = pvs.tile([DH + 1, S], F32, tag="pv", name=f"pv{h0}")
                pv1 = pvs.tile([DH + 1, S], F32, tag="pv", name=f"pv{h1}")

                def emit_pv(c, et0, et1):
                    for pv, h, et in ((pv0, h0, et0), (pv1, h1, et1)):
                        for nh in range(NQ):
                            nc.tensor.matmul(
                                pv[:, 512 * nh: 512 * nh + 512],
                                allv[:, h, c, :],
                                et[:, 512 * nh: 512 * nh + 512],
                                start=(c == 0), stop=(c == KCH - 1),
                            )

                prev = None
                for c in range(KCH):
                    self_side = c < SC
                    kt_src = pt["k" if self_side else "ko"][p]
                    qt_src = pt["q" if self_side else "qo"][p]
                    col = 128 * (c % SC)
                    sc0 = sps.tile([128, S], F32, tag="sc", name=f"sc{h0}_{c}")
                    sc1 = sps.tile([128, S], F32, tag="sc", name=f"sc{h1}_{c}")
                    # Adjacent K=64 matmuls on row groups 0 / 64 overlap in
                    # the PE array.
                    for nh in range(NQ):
                        nc.tensor.matmul(
                            sc0[:, 512 * nh: 512 * nh + 512],
                            kt_src[0:64, col:col + 128],
                            qt_src[0:64, 512 * nh: 512 * nh + 512],
                            start=True, stop=True,
                        )
                        nc.tensor.matmul(
                            sc1[:, 512 * nh: 512 * nh + 512],
                            kt_src[64:128, col:col + 128],
                            qt_src[64:128, 512 * nh: 512 * nh + 512],
                            start=True, stop=True,
                        )
                    bias = (mact_t[:, (c % SC):(c % SC) + 1] if self_side
                            else mzero_t[:])
                    et0 = expp.tile([128, S], F16, tag="et", name=f"et{h0}_{c}")
                    nc.scalar.activation(et0[:], sc0[:], AF.Exp, bias=bias,
                                         scale=0.125)
                    et1 = expp.tile([128, S], F16, tag="et", name=f"et{h1}_{c}")
                    nc.scalar.activation(et1[:], sc1[:], AF.Exp, bias=bias,
                                         scale=0.125)
                    if fillers and c >= 1:
                        fillers.pop(0)()
                    # PV of the previous chunk: the PE waits on exp(c-1) only
                    # after issuing chunk c's scores, absorbing exp latency.
                    if prev is not None:
                        emit_pv(*prev)
                    prev = (c, et0, et1)
                emit_pv(*prev)
                for f in fillers:
                    f()
                cts = []
                for h, pv in ((h0, pv0), (h1, pv1)):
                    ct = ctxp.tile([DH + 1, S], F16, tag="ct", name=f"ct{h}")
                    nc.vector.tensor_copy(ct[:], pv[:])
                    cts.append((h, ct))

                def tail():
                    for h, ct in cts:
                        for qc in range(SC):
                            tp = sps.tile([128, DH + 1], F16, tag="sc",
                                          name=f"tp{h}_{qc}")
                            nc.tensor.transpose(
                                tp[:], ct[:, 128 * qc: 128 * qc + 128],
                                eye_t[0:DH + 1, 0:DH + 1],
                            )
                            rec = smal.tile([128, 1], F32, tag="rec",
                                            name=f"rec{h}_{qc}")
                            nc.vector.reciprocal(rec[:], tp[:, DH:DH + 1])
                            nc.vector.tensor_scalar_mul(
                                out_sb[p][qc][:, DH * (h % 2):
                                              DH * (h % 2) + DH],
                                tp[:, 0:DH], rec[:])
                    for qc in range(SC):
                        nc.sync.dma_start(
                            out_d[128 * qc: 128 * qc + 128,
                                  128 * p: 128 * p + 128],
                            out_sb[p][qc][:])
                return tail

            def pair_fillers(p):
                return [
                    (lambda ty=ty, nh=nh: projT_group(ty, p, nh))
                    for ty in ("q", "k", "qo", "ko") for nh in range(NQ)
                ]

            projT("q", 0)
            projT("k", 0)
            projV(0)
            projT("qo", 0)
            projT("ko", 0)
            # The other-stream value projection and the later pairs'
            # transposed projections ride as attention fillers: vo chunk j
            # lands at filler slot j+1, well before PV chunk 8+j reads it.
            vo_fillers = [(lambda sc=sc: projV_group(1, sc))
                          for sc in range(SC)]
            t0 = attention(0, vo_fillers + pair_fillers(1))
            t1 = attention(1, [t0] + pair_fillers(2))
            t2 = attention(2, [t1])
            t2()

    return nc


def _to_chunked(a, ncols):
    """[KC*128, ncols] -> [128, KC*ncols] with chunk c at cols [c*ncols, ...)."""
    return np.ascontiguousarray(
        a.reshape(KC, 128, ncols).transpose(1, 0, 2).reshape(128, KC * ncols)
    )


def _shard_inputs(hidden_states, hidden_states_other, attention_mask,
                  Wq, bq, Wk, bk, Wv, bv, Wqo, bqo, Wko, bko, Wvo, bvo):
    f32, f16 = np.float32, np.float16
    hs = np.asarray(hidden_states, f32)
    hso = np.asarray(hidden_states_other, f32)
    am = np.asarray(attention_mask, f32)
    ws = {"q": (Wq, bq), "k": (Wk, bk), "qo": (Wqo, bqo), "ko": (Wko, bko),
          "v": (Wv, bv), "vo": (Wvo, bvo)}
    ws = {ty: (np.asarray(w, f32), np.asarray(b, f32))
          for ty, (w, b) in ws.items()}

    eye = np.eye(128, dtype=f16)
    mzero = np.zeros((128, 1), f32)

    in_maps = []
    for core in range(N_CORES):
        b, hh = core // 2, core % 2
        m = {
            "xt": _to_chunked(hs[b].T, S).astype(f16),
            "xot": _to_chunked(hso[b].T, S).astype(f16),
        }
        sl = slice(MW * hh, MW * hh + MW)
        pbias = np.zeros((128, 12), f32)
        for ty, (W, bias) in ws.items():
            m[f"w{ty}"] = _to_chunked(W[sl].T, MW).astype(f16)
            if ty in ("q", "k", "qo", "ko"):
                col0 = 3 * {"q": 0, "k": 1, "qo": 2, "ko": 3}[ty]
                for p in range(3):
                    pbias[:, col0 + p] = bias[sl][128 * p: 128 * p + 128]
        m["pbias"] = pbias
        vb = np.concatenate([ws["v"][1][sl], ws["vo"][1][sl]])
        m["vbias"] = np.broadcast_to(vb, (128, 2 * MW)).copy()
        m["mact"] = np.ascontiguousarray(am[b, 0, 0].reshape(SC, 128).T)
        m["mzero"] = mzero
        m["eye"] = eye
        in_maps.append(m)
    return in_maps


_NC_CACHE = {}


def _get_nc(repeat=1):
    if repeat not in _NC_CACHE:
        _NC_CACHE[repeat] = _build(repeat)
    return _NC_CACHE[repeat]


def kernel(**inputs):
    in_maps = _shard_inputs(**inputs)
    nc = _get_nc()
    res = run_bass_kernel_spmd(nc, in_maps, core_ids=list(range(N_CORES)))
    out = np.empty((B, S, D), np.float32)
    for core in range(N_CORES):
        b, hh = core // 2, core % 2
        out[b, :, MW * hh:MW * hh + MW] = res.results[core]["out"]
    return out
